# revision 3
# baseline (speedup 1.0000x reference)
"""Multi-head attention (B=2, S=2048, D=1024, H=16) on 8 TRN2 NeuronCores.

Sharding (Megatron-style): heads are tensor-parallel across the 8 cores
(2 heads each, batch replicated). Wq/Wk/Wv are column-parallel (each core
gets its heads' 128 output rows), Wo is row-parallel (each core gets the
matching 128 input columns); each core computes a full-shape fp16 partial
of the output projection and the host sums the 8 partials (the
row-parallel all-reduce, done at unshard time).

Per-core kernel (fp16 operands, fp32 PSUM accumulation), written as one
flat tile program so the Tile scheduler overlaps batch b+1's projections
with batch b's attention, and the scalar-engine exp stream (the
throughput limit) starts as early as possible:

  QT/KT = (x @ W.T).T computed directly in [head-dim, seq] layout
  V transposed to [seq, head-dim] via PE transpose, augmented with a ones
    column so the PV matmul also produces the softmax denominator
  S_T   = K_block.T @ Q per 128-key block, both heads co-issued on
          disjoint PE row groups (K=64 each) via tile_position
  P_T   = exp(0.125 * S_T) on the scalar engine (scores are ~N(0,1), so
          no max-subtraction is needed)
  O_aug = V_aug.T @ P_T accumulated over key blocks ([65, 512]; row 64 is
          the denominator)
  y     = O_aug[0:64] * broadcast(1/denominator)
  out  += y_block.T @ Wo_slice.T  (fp16 partial, summed on host)

PSUM budget (8 banks): shared "mm" tag (q/k/v proj, V-transpose, out-proj
tiles) x2 bufs = 2 banks; score tiles [128,2,512]f32 x2 bufs = 4 banks;
two PV accumulators = 2 banks.
"""

from contextlib import ExitStack

import numpy as np

import concourse.bass as bass
import concourse.mybir as mybir
import concourse.tile as tile
from concourse import bacc
from concourse.masks import make_identity

F32 = mybir.dt.float32
F16 = mybir.dt.float16

B = 2
S = 2048
D = 1024
H_LOCAL = 2          # heads per core
BS = B * S           # 4096
NE = D // 128        # contraction tiles for the projections
CHUNK = 512          # query-chunk width
NCH = S // CHUNK     # chunks per batch element
NTB = S // 128       # key blocks per batch element
SCALE = 0.125        # 1/sqrt(head_dim)
N_CORES = 8


def _proj_chunk(nc, pools, w_sb, ident, qT, kT, v_aug, xT, b, c):
    """Project one 512-token chunk of batch b: q/k rows into qT/kT and
    transposed V blocks into v_aug."""
    x_pool, vt_pool, mm_ps = pools["x"], pools["vt"], pools["mm_ps"]
    g = b * NCH + c
    cols = bass.ds(g * CHUNK, CHUNK)

    xt = x_pool.tile([128, NE, CHUNK], F16, tag="xt", name="xt")
    for e in range(NE):
        nc.sync.dma_start(out=xt[:, e, :], in_=xT[e * 128:(e + 1) * 128, cols])

    ps = {}
    for name in ("wq", "wk", "wv"):
        p = mm_ps.tile([128, CHUNK], F32, tag="mm", name=f"ps_{name}")
        for e in range(NE):
            nc.tensor.matmul(p[:], w_sb[name][:, e, :], xt[:, e, :],
                             start=(e == 0), stop=(e == NE - 1))
        ps[name] = p
    nc.any.tensor_copy(qT[:, cols], ps["wq"][:])
    nc.any.tensor_copy(kT[:, cols], ps["wk"][:])
    vt = vt_pool.tile([128, CHUNK], F16, tag="vt", name="vt")
    nc.any.tensor_copy(vt[:], ps["wv"][:])

    # Transpose V [head-dim, tok] -> [tok, head-dim] per 128-token block;
    # both heads co-issued on disjoint 64-row PE tiles. Each head gets its
    # own PSUM tile (separate banks — the co-issued pair must not write the
    # same bank), then one strided copy per head into v_aug.
    tr = [mm_ps.tile([128, 4, 64], F16, tag="mm", name=f"tr{h}")
          for h in range(H_LOCAL)]
    for j in range(CHUNK // 128):
        for h in range(H_LOCAL):
            nc.tensor.transpose(tr[h][:, j, :], vt[64 * h:64 * h + 64,
                                                   bass.ds(j * 128, 128)],
                                ident[64 * h:64 * h + 64, 0:64])
    for h in range(H_LOCAL):
        nc.vector.tensor_copy(
            v_aug[:, b * H_LOCAL + h, bass.ds(c * 4, 4), 0:64],
            tr[h][:, :, :])


def _attn_chunk(nc, pools, wo_sb, qT, kT, v_aug, y_cT, out, b, c):
    """Attention for one 512-query chunk of batch b, then the output
    projection + DMA for its four 128-token blocks."""
    pt_pool, nrm_pool = pools["pt"], pools["nrm"]
    out_pool, mm_ps, sc_ps, o_ps = (pools["out"], pools["mm_ps"],
                                    pools["sc_ps"], pools["o_ps"])
    scols = bass.ds(b * S + c * CHUNK, CHUNK)

    o = {}
    for h in range(H_LOCAL):
        o[h] = o_ps.tile([65, CHUNK], F32, tag=f"o{h}", name=f"o{h}")
    for t in range(NTB):
        tcols = bass.ds(b * S + t * 128, 128)
        sc = sc_ps.tile([128, H_LOCAL, CHUNK], F32, tag="sc", name="sc")
        for h in range(H_LOCAL):
            hp = slice(64 * h, 64 * h + 64)
            nc.tensor.matmul(sc[:, h, :], kT[hp, tcols], qT[hp, scols],
                             start=True, stop=True,
                             tile_position=(64 * h, 0))
        pt = pt_pool.tile([128, H_LOCAL, CHUNK], F16, tag="pt", name="pt")
        nc.scalar.activation(pt[:], sc[:],
                             mybir.ActivationFunctionType.Exp, scale=SCALE)
        for h in range(H_LOCAL):
            nc.tensor.matmul(o[h][:], v_aug[:, b * H_LOCAL + h, t, :],
                             pt[:, h, :],
                             start=(t == 0), stop=(t == NTB - 1))

    for h in range(H_LOCAL):
        rs = nrm_pool.tile([1, CHUNK], F32, tag="rs", name="rs")
        nc.vector.tensor_copy(rs[:], o[h][64:65, :])
        bc = nrm_pool.tile([64, CHUNK], F32, tag="bc", name="bc")
        nc.gpsimd.partition_broadcast(bc[:], rs[:])
        bcr = nrm_pool.tile([64, CHUNK], F32, tag="bcr", name="bcr")
        nc.vector.reciprocal_approx_fast(out=bcr[:], in_=bc[:])
        nc.vector.tensor_mul(y_cT[64 * h:64 * h + 64, scols],
                             o[h][0:64, :], bcr[:])

    for j in range(CHUNK // 128):
        rows = bass.ds(b * S + (c * 4 + j) * 128, 128)
        ot = out_pool.tile([128, D], F16, tag="ot", name="ot")
        for f in range(D // CHUNK):
            fcols = bass.ds(f * CHUNK, CHUNK)
            po = mm_ps.tile([128, CHUNK], F32, tag="mm", name="po")
            nc.tensor.matmul(po[:], y_cT[:, rows], wo_sb[:, fcols],
                             start=True, stop=True)
            nc.any.tensor_copy(ot[:, fcols], po[:])
        nc.sync.dma_start(out=out[rows, :], in_=ot[:])


def _mha_kernel(tc, out, xT, wqT, wkT, wvT, woT):
    nc = tc.nc
    with ExitStack() as ctx:
        singles = ctx.enter_context(tc.tile_pool(name="singles", bufs=1))

        w_sb = {}
        for name, ap in (("wq", wqT), ("wk", wkT), ("wv", wvT)):
            t = singles.tile([128, NE, 128], F16, tag=f"w_{name}",
                             name=f"w_{name}")
            nc.sync.dma_start(out=t[:],
                              in_=ap.rearrange("(e p) o -> p e o", p=128))
            w_sb[name] = t
        wo_sb = singles.tile([128, D], F16, tag="wo")
        nc.sync.dma_start(out=wo_sb[:], in_=woT[:])

        # 64x64 identity in both partition halves so the PE-transpose's
        # identity operand matches the input's base partition.
        ident = singles.tile([128, 64], F16, tag="ident")
        make_identity(nc, ident[0:64, 0:64])
        make_identity(nc, ident[64:128, 0:64])

        qT = singles.tile([128, BS], F16, tag="qT")
        kT = singles.tile([128, BS], F16, tag="kT")
        v_aug = singles.tile([128, B * H_LOCAL, NTB, 65], F16, tag="v_aug")
        ones = singles.tile([128, 1], F16, tag="ones")
        nc.vector.memset(ones[:], 1.0)
        nc.vector.tensor_copy(
            v_aug[:, :, :, 64:65],
            ones[:].to_broadcast((128, B * H_LOCAL, NTB, 1)))
        y_cT = singles.tile([128, BS], F16, tag="y_cT")

        pools = {
            "x": ctx.enter_context(tc.tile_pool(name="x_pool", bufs=4)),
            "vt": ctx.enter_context(tc.tile_pool(name="vt_pool", bufs=2)),
            "pt": ctx.enter_context(tc.tile_pool(name="pt_pool", bufs=3)),
            "nrm": ctx.enter_context(tc.tile_pool(name="nrm_pool", bufs=2)),
            "out": ctx.enter_context(tc.tile_pool(name="out_pool", bufs=3)),
            "mm_ps": ctx.enter_context(
                tc.tile_pool(name="mm_ps", bufs=2, space="PSUM")),
            "sc_ps": ctx.enter_context(
                tc.tile_pool(name="sc_ps", bufs=2, space="PSUM")),
            "o_ps": ctx.enter_context(
                tc.tile_pool(name="o_ps", bufs=1, space="PSUM")),
        }

        for b in range(B):
            for c in range(NCH):
                _proj_chunk(nc, pools, w_sb, ident, qT, kT, v_aug, xT, b, c)
            for c in range(NCH):
                with nc.named_scope(f"attn_b{b}c{c}"):
                    _attn_chunk(nc, pools, wo_sb, qT, kT, v_aug, y_cT,
                                out, b, c)


def build_nc(n_cores=N_CORES):
    nc = bacc.Bacc("TRN2", target_bir_lowering=False, debug=False,
                   num_devices=n_cores)
    xT = nc.dram_tensor("xT", [D, BS], F16, kind="ExternalInput").ap()
    wqT = nc.dram_tensor("wqT", [D, 128], F16, kind="ExternalInput").ap()
    wkT = nc.dram_tensor("wkT", [D, 128], F16, kind="ExternalInput").ap()
    wvT = nc.dram_tensor("wvT", [D, 128], F16, kind="ExternalInput").ap()
    woT = nc.dram_tensor("woT", [128, D], F16, kind="ExternalInput").ap()
    out = nc.dram_tensor("out", [BS, D], F16, kind="ExternalOutput").ap()
    with tile.TileContext(nc) as tc:
        _mha_kernel(tc, out, xT, wqT, wkT, wvT, woT)
    nc.compile()
    return nc


def make_in_maps(inputs, Wq, Wk, Wv, Wo, n_cores=N_CORES):
    x = np.asarray(inputs, dtype=np.float32).reshape(BS, D)
    xT = np.ascontiguousarray(x.T).astype(np.float16)
    Wq, Wk, Wv, Wo = (np.asarray(w, dtype=np.float32)
                      for w in (Wq, Wk, Wv, Wo))
    maps = []
    for c in range(n_cores):
        sl = slice(c * 128, (c + 1) * 128)
        maps.append({
            "xT": xT,
            "wqT": np.ascontiguousarray(Wq[sl, :].T).astype(np.float16),
            "wkT": np.ascontiguousarray(Wk[sl, :].T).astype(np.float16),
            "wvT": np.ascontiguousarray(Wv[sl, :].T).astype(np.float16),
            "woT": np.ascontiguousarray(Wo[:, sl].T).astype(np.float16),
        })
    return maps


_NC_CACHE = None


def run(inputs, Wq, Wk, Wv, Wo, trace=False):
    """Shard, run on the 8 NeuronCores, and unshard. Returns
    (output [B,S,D] float32, BassKernelResults)."""
    global _NC_CACHE
    from concourse.bass_utils import run_bass_kernel_spmd
    if _NC_CACHE is None:
        _NC_CACHE = build_nc()
    maps = make_in_maps(inputs, Wq, Wk, Wv, Wo)
    res = run_bass_kernel_spmd(_NC_CACHE, maps, list(range(N_CORES)),
                               trace=trace)
    acc = np.zeros((BS, D), dtype=np.float32)
    for rmap in res.results:
        acc += rmap["out"].astype(np.float32)
    return acc.reshape(B, S, D), res


def kernel(inputs, Wq, Wk, Wv, Wo):
    out, _ = run(inputs, Wq, Wk, Wv, Wo, trace=False)
    return out


# revision 7
# speedup vs baseline: 1.0448x; 1.0448x over previous
"""Multi-head attention (B=2, S=2048, D=1024, H=16) on 8 TRN2 NeuronCores.

Sharding (Megatron-style): heads are tensor-parallel across the 8 cores
(2 heads each, batch replicated). Wq/Wk/Wv are column-parallel (each core
gets its heads' 128 output rows), Wo is row-parallel (each core gets the
matching 128 input columns); each core computes a full-shape fp16 partial
of the output projection and the host sums the 8 partials (the
row-parallel all-reduce, done at unshard time).

Per-core kernel (fp16 operands, fp32 PSUM accumulation), written as one
flat tile program so the Tile scheduler overlaps batch b+1's projections
with batch b's attention, and the scalar-engine exp stream (the
throughput limit) starts as early as possible:

  QT/KT = (x @ W.T).T computed directly in [head-dim, seq] layout
  V transposed to [seq, head-dim] via PE transpose, augmented with a ones
    column so the PV matmul also produces the softmax denominator
  S_T   = K_block.T @ Q per 128-key block, both heads co-issued on
          disjoint PE row groups (K=64 each) via tile_position
  P_T   = exp(0.125 * S_T) on the scalar engine (scores are ~N(0,1), so
          no max-subtraction is needed)
  O_aug = V_aug.T @ P_T accumulated over key blocks ([65, 512]; row 64 is
          the denominator)
  y     = O_aug[0:64] * broadcast(1/denominator)
  out  += y_block.T @ Wo_slice.T  (fp16 partial, summed on host)

PSUM budget (8 banks): shared "mm" tag (q/k/v proj, V-transpose, out-proj
tiles) x2 bufs = 2 banks; score tiles [128,2,512]f32 x2 bufs = 4 banks;
two PV accumulators = 2 banks.
"""

from contextlib import ExitStack

import numpy as np

import concourse.bass as bass
import concourse.mybir as mybir
import concourse.tile as tile
from concourse import bacc
from concourse.masks import make_identity

F32 = mybir.dt.float32
F16 = mybir.dt.float16

B = 2
S = 2048
D = 1024
H_LOCAL = 2          # heads per core
BS = B * S           # 4096
NE = D // 128        # contraction tiles for the projections
CHUNK = 512          # query-chunk width
NCH = S // CHUNK     # chunks per batch element
NTB = S // 128       # key blocks per batch element
SCALE = 0.125        # 1/sqrt(head_dim)
N_CORES = 8


def _proj_chunk(nc, pools, w_sb, ident, qT, kT, v_aug, xT, b, c):
    """Project one 512-token chunk of batch b: q/k rows into qT/kT and
    transposed V blocks into v_aug."""
    x_pool, vt_pool, mm_ps = pools["x"], pools["vt"], pools["mm_ps"]
    g = b * NCH + c
    cols = bass.ds(g * CHUNK, CHUNK)

    xt = x_pool.tile([128, NE, CHUNK], F16, tag="xt", name="xt")
    for e in range(NE):
        nc.sync.dma_start(out=xt[:, e, :], in_=xT[e * 128:(e + 1) * 128, cols])

    ps = {}
    for name in ("wq", "wk", "wv"):
        p = mm_ps.tile([128, CHUNK], F32, tag="mm", name=f"ps_{name}")
        for e in range(NE):
            nc.tensor.matmul(p[:], w_sb[name][:, e, :], xt[:, e, :],
                             start=(e == 0), stop=(e == NE - 1))
        ps[name] = p
    nc.vector.tensor_copy(qT[:, cols], ps["wq"][:])
    nc.vector.tensor_copy(kT[:, cols], ps["wk"][:])
    vt = vt_pool.tile([128, CHUNK], F16, tag="vt", name="vt")
    nc.vector.tensor_copy(vt[:], ps["wv"][:])

    # Transpose V [head-dim, tok] -> [tok, head-dim] per 128-token block;
    # both heads co-issued on disjoint 64-row PE tiles. Each head gets its
    # own PSUM tile (separate banks — the co-issued pair must not write the
    # same bank), then one strided copy per head into v_aug.
    tr = [mm_ps.tile([128, 4, 64], F16, tag="mm", name=f"tr{h}")
          for h in range(H_LOCAL)]
    for j in range(CHUNK // 128):
        for h in range(H_LOCAL):
            nc.tensor.transpose(tr[h][:, j, :], vt[64 * h:64 * h + 64,
                                                   bass.ds(j * 128, 128)],
                                ident[64 * h:64 * h + 64, 0:64])
    for h in range(H_LOCAL):
        nc.vector.tensor_copy(
            v_aug[:, b * H_LOCAL + h, bass.ds(c * 4, 4), 0:64],
            tr[h][:, :, :])


def _attn_chunk(nc, pools, wo_sb, qT, kT, v_aug, y_cT, out, b, c):
    """Attention for one 512-query chunk of batch b, then the output
    projection + DMA for its four 128-token blocks."""
    pt_pool, nrm_pool = pools["pt"], pools["nrm"]
    out_pool, mm_ps, sc_ps, o_ps = (pools["out"], pools["mm_ps"],
                                    pools["sc_ps"], pools["o_ps"])
    scols = bass.ds(b * S + c * CHUNK, CHUNK)

    o = {}
    for h in range(H_LOCAL):
        o[h] = o_ps.tile([65, CHUNK], F32, tag=f"o{h}", name=f"o{h}")
    for t in range(NTB):
        tcols = bass.ds(b * S + t * 128, 128)
        sc = sc_ps.tile([128, H_LOCAL, CHUNK], F32, tag="sc", name="sc")
        for h in range(H_LOCAL):
            hp = slice(64 * h, 64 * h + 64)
            nc.tensor.matmul(sc[:, h, :], kT[hp, tcols], qT[hp, scols],
                             start=True, stop=True,
                             tile_position=(64 * h, 0))
        pt = pt_pool.tile([128, H_LOCAL, CHUNK], F16, tag="pt", name="pt")
        nc.scalar.activation(pt[:], sc[:],
                             mybir.ActivationFunctionType.Exp, scale=SCALE)
        for h in range(H_LOCAL):
            nc.tensor.matmul(o[h][:], v_aug[:, b * H_LOCAL + h, t, :],
                             pt[:, h, :],
                             start=(t == 0), stop=(t == NTB - 1))

    for h in range(H_LOCAL):
        rs = nrm_pool.tile([1, CHUNK], F32, tag="rs", name="rs")
        nc.vector.tensor_copy(rs[:], o[h][64:65, :])
        rr = nrm_pool.tile([1, CHUNK], F32, tag="rr", name="rr")
        nc.vector.reciprocal_approx_fast(out=rr[:], in_=rs[:])
        bcr = nrm_pool.tile([64, CHUNK], F32, tag="bcr", name="bcr")
        nc.gpsimd.partition_broadcast(bcr[:], rr[:])
        nc.vector.tensor_mul(y_cT[64 * h:64 * h + 64, scols],
                             o[h][0:64, :], bcr[:])

    for j in range(CHUNK // 128):
        rows = bass.ds(b * S + (c * 4 + j) * 128, 128)
        ot = out_pool.tile([128, D], F16, tag="ot", name="ot")
        for f in range(D // CHUNK):
            fcols = bass.ds(f * CHUNK, CHUNK)
            po = mm_ps.tile([128, CHUNK], F32, tag="mm", name="po")
            nc.tensor.matmul(po[:], y_cT[:, rows], wo_sb[:, fcols],
                             start=True, stop=True)
            nc.vector.tensor_copy(ot[:, fcols], po[:])
        nc.sync.dma_start(out=out[rows, :], in_=ot[:])


def _mha_kernel(tc, out, xT, wqT, wkT, wvT, woT):
    nc = tc.nc
    with ExitStack() as ctx:
        singles = ctx.enter_context(tc.tile_pool(name="singles", bufs=1))

        w_sb = {}
        for name, ap in (("wq", wqT), ("wk", wkT), ("wv", wvT)):
            t = singles.tile([128, NE, 128], F16, tag=f"w_{name}",
                             name=f"w_{name}")
            nc.sync.dma_start(out=t[:],
                              in_=ap.rearrange("(e p) o -> p e o", p=128))
            w_sb[name] = t
        wo_sb = singles.tile([128, D], F16, tag="wo")
        nc.sync.dma_start(out=wo_sb[:], in_=woT[:])

        # 64x64 identity in both partition halves so the PE-transpose's
        # identity operand matches the input's base partition.
        ident = singles.tile([128, 64], F16, tag="ident")
        make_identity(nc, ident[0:64, 0:64])
        make_identity(nc, ident[64:128, 0:64])

        qT = singles.tile([128, BS], F16, tag="qT")
        kT = singles.tile([128, BS], F16, tag="kT")
        v_aug = singles.tile([128, B * H_LOCAL, NTB, 65], F16, tag="v_aug")
        ones = singles.tile([128, 1], F16, tag="ones")
        nc.vector.memset(ones[:], 1.0)
        nc.vector.tensor_copy(
            v_aug[:, :, :, 64:65],
            ones[:].to_broadcast((128, B * H_LOCAL, NTB, 1)))
        y_cT = singles.tile([128, BS], F16, tag="y_cT")

        pools = {
            "x": ctx.enter_context(tc.tile_pool(name="x_pool", bufs=4)),
            "vt": ctx.enter_context(tc.tile_pool(name="vt_pool", bufs=2)),
            "pt": ctx.enter_context(tc.tile_pool(name="pt_pool", bufs=8)),
            "nrm": ctx.enter_context(tc.tile_pool(name="nrm_pool", bufs=2)),
            "out": ctx.enter_context(tc.tile_pool(name="out_pool", bufs=3)),
            "mm_ps": ctx.enter_context(
                tc.tile_pool(name="mm_ps", bufs=2, space="PSUM")),
            "sc_ps": ctx.enter_context(
                tc.tile_pool(name="sc_ps", bufs=2, space="PSUM")),
            "o_ps": ctx.enter_context(
                tc.tile_pool(name="o_ps", bufs=1, space="PSUM")),
        }

        for b in range(B):
            for c in range(NCH):
                _proj_chunk(nc, pools, w_sb, ident, qT, kT, v_aug, xT, b, c)
            for c in range(NCH):
                with nc.named_scope(f"attn_b{b}c{c}"):
                    _attn_chunk(nc, pools, wo_sb, qT, kT, v_aug, y_cT,
                                out, b, c)


def build_nc(n_cores=N_CORES):
    nc = bacc.Bacc("TRN2", target_bir_lowering=False, debug=False,
                   num_devices=n_cores)
    xT = nc.dram_tensor("xT", [D, BS], F16, kind="ExternalInput").ap()
    wqT = nc.dram_tensor("wqT", [D, 128], F16, kind="ExternalInput").ap()
    wkT = nc.dram_tensor("wkT", [D, 128], F16, kind="ExternalInput").ap()
    wvT = nc.dram_tensor("wvT", [D, 128], F16, kind="ExternalInput").ap()
    woT = nc.dram_tensor("woT", [128, D], F16, kind="ExternalInput").ap()
    out = nc.dram_tensor("out", [BS, D], F16, kind="ExternalOutput").ap()
    with tile.TileContext(nc) as tc:
        _mha_kernel(tc, out, xT, wqT, wkT, wvT, woT)
    nc.compile()
    return nc


def make_in_maps(inputs, Wq, Wk, Wv, Wo, n_cores=N_CORES):
    x = np.asarray(inputs, dtype=np.float32).reshape(BS, D)
    xT = np.ascontiguousarray(x.T).astype(np.float16)
    Wq, Wk, Wv, Wo = (np.asarray(w, dtype=np.float32)
                      for w in (Wq, Wk, Wv, Wo))
    maps = []
    for c in range(n_cores):
        sl = slice(c * 128, (c + 1) * 128)
        maps.append({
            "xT": xT,
            "wqT": np.ascontiguousarray(Wq[sl, :].T).astype(np.float16),
            "wkT": np.ascontiguousarray(Wk[sl, :].T).astype(np.float16),
            "wvT": np.ascontiguousarray(Wv[sl, :].T).astype(np.float16),
            "woT": np.ascontiguousarray(Wo[:, sl].T).astype(np.float16),
        })
    return maps


_NC_CACHE = None


def run(inputs, Wq, Wk, Wv, Wo, trace=False):
    """Shard, run on the 8 NeuronCores, and unshard. Returns
    (output [B,S,D] float32, BassKernelResults)."""
    global _NC_CACHE
    from concourse.bass_utils import run_bass_kernel_spmd
    if _NC_CACHE is None:
        _NC_CACHE = build_nc()
    maps = make_in_maps(inputs, Wq, Wk, Wv, Wo)
    res = run_bass_kernel_spmd(_NC_CACHE, maps, list(range(N_CORES)),
                               trace=trace)
    acc = np.zeros((BS, D), dtype=np.float32)
    for rmap in res.results:
        acc += rmap["out"].astype(np.float32)
    return acc.reshape(B, S, D), res


def kernel(inputs, Wq, Wk, Wv, Wo):
    out, _ = run(inputs, Wq, Wk, Wv, Wo, trace=False)
    return out
